# revision 20
# baseline (speedup 1.0000x reference)
"""Chamfer distance kernel for Trainium2 (8 NeuronCores, data-parallel over batch).

Input : x, y float32 [16, 4096, 3]
Output: scalar float32 = mean_b [ mean_n min_m ||x_bn - y_bm||^2
                                + mean_m min_n ||x_bn - y_bm||^2 ]

This environment charges a large, roughly flat cost per *instruction*
(engines do not overlap), so the kernel minimizes instruction count by
computing squared distances directly on the Vector engine with giant
multi-dim access patterns (128 x 49152 elements per op, stride-0
broadcasts), instead of PE matmuls (capped at 512 columns per
instruction, which would need 512+ instructions per core).

Per core (2 batches, 32 x-blocks of 128 rows each, groups of G=4 blocks):
  c[p,k,g,m] = x_k[blk g, row p] - y_k[m]      1 TT sub (4D broadcast AP)
  u = c0^2 + c1^2 + c2^2                       2 custom DVE ops (SQSQ, ADDSQ)
  rowacc[p, blk] = min_m u                     1 segmented reduce
  colstack[g] = min over the G blocks          2 TT folds
After 8 groups: fold colstack 8->1 (3 TT), DMA out per-batch column mins
[128, 4096] f16 and row mins [128, 64] f32. The host does the final
128-partition min + mean (tiny numpy) because cross-partition DVE inputs
are illegal on this target and a DMA out is 1 instruction vs a 14-op tree.
y coordinates are loaded via a partition-broadcast DMA (24 KB upload per
batch instead of 3 MB).
"""
import sys

sys.path.insert(0, "/opt/trn_rl_repo")

import numpy as np

import concourse.bacc as bacc
import concourse.tile as tile
from concourse import mybir
from concourse.alu_op_type import AluOpType
from concourse.bass_utils import run_bass_kernel_spmd

# --- custom DVE ops (registered at import time) ---------------------------
import concourse.dve_ops as dve_ops
from concourse.dve_ops import DveOp
from concourse.dve_spec import Spec, Src0, Src1, sq, lower, _has_src1


def _register_dve_op(name, spec):
    if name in dve_ops._SUB_OPCODE_FOR_NAME:
        for o in dve_ops.OPS:
            if o.name == name:
                return o
    row = dve_ops._CUSTOM_DVE_ROW_BASE + len(dve_ops.OPS)
    assert row < 0x20
    dve_ops._SUB_OPCODE_FOR_NAME[name] = row
    from concourse.dve_uop import DveOpSpec

    shas = {}
    for ver in ("v3", "v4"):
        try:
            uops = lower(spec, ver=ver)
            s = DveOpSpec(name=name, opcode=row, uops=uops, rd1_en=_has_src1(spec))
            shas[ver] = s.sha(ver)
        except Exception:
            pass
    op = DveOp(name, spec, subdim=False, uops_sha=shas)
    dve_ops.OPS.append(op)
    dve_ops.CUSTOM_DVE_SPECS[name] = spec
    return op


SQSQ = _register_dve_op(
    "SQSQ_ANT",
    Spec(
        body=sq(Src0) + sq(Src1),
        reference=lambda in0, in1, s0, s1, imm2: (
            in0.astype(np.float32) ** 2 + in1.astype(np.float32) ** 2
        ),
    ),
)
ADDSQ = _register_dve_op(
    "ADDSQ_ANT",
    Spec(
        body=Src0 + sq(Src1),
        reference=lambda in0, in1, s0, s1, imm2: (
            in0.astype(np.float32) + in1.astype(np.float32) ** 2
        ),
    ),
)
# ---------------------------------------------------------------------------

F32 = mybir.dt.float32
F16 = mybir.dt.float16
X = mybir.AxisListType.X
A = AluOpType

B, N, KC = 16, 4096, 3
NCORES = 8
BPC = B // NCORES            # batches per core
NBLK = N // 128              # 32 x-blocks per batch
G = 8                        # blocks per group
NG = NBLK // G               # 4 groups per batch
GS = 2 * N + 8               # per-g stride (pad 8 stops full AP merging, which
                             # would overflow the 16-bit ISA num_elem field)
CW = G * GS                  # c[p, g*GS + m*2 + k], k-interleaved pairs so the
                             # big subtract runs with step-1 innermost (2x mode)


def _build_nc(repeat: int = 1):
    nc = bacc.Bacc("TRN2", target_bir_lowering=False, debug=False, num_devices=NCORES)
    xp_d = nc.dram_tensor("xp", [128, BPC * NBLK * KC], F16, kind="ExternalInput").ap()
    yp_ds = [
        nc.dram_tensor(f"yp{b}", [1, N * KC], F16, kind="ExternalInput").ap()
        for b in range(BPC)
    ]
    col_d = nc.dram_tensor("col", [128, BPC * N], F16, kind="ExternalOutput").ap()
    row_d = nc.dram_tensor("row", [128, BPC * NBLK], F32, kind="ExternalOutput").ap()

    with tile.TileContext(nc) as tc:
        import contextlib
        with contextlib.ExitStack() as ctx:
            const = ctx.enter_context(tc.tile_pool(name="const", bufs=1))

            xp_t = const.tile([128, BPC * NBLK * KC], F16, name="xp_t")
            nc.gpsimd.dma_start(xp_t[:], xp_d[:])
            yp_t = const.tile([128, N * KC], F16, name="yp_t")
            c_t = const.tile([128, CW], F16, name="c_t")
            cs_t = const.tile([128, NG * N], F16, name="cs_t")
            colout = const.tile([128, BPC * N], F16, name="colout")
            rowacc = const.tile([128, BPC * NBLK], F32, name="rowacc")

            # c layout [p, g, m, k(2)] with per-g pad; sub iterates [p, g, m, k]
            cgmk = c_t[:].rearrange("p (g q) -> p g q", g=G)[:, :, 0:2 * N] \
                         .rearrange("p g (m k) -> p g m k", k=2)
            u_v = cgmk[:, :, :, 0]                    # [128, G, N] k=0 slots
            v_v = cgmk[:, :, :, 1]                    # [128, G, N] k=1 slots
            cB = cgmk[:, :, :, 1:2]
            ymk = yp_t[:].rearrange("p (m k) -> p m k", k=KC)
            y_apA = ymk[:, :, 0:2].unsqueeze(1).broadcast_to([128, G, N, 2])
            y_apB = ymk[:, :, 2:3].unsqueeze(1).broadcast_to([128, G, N, 1])

            for _rep in range(repeat):
                for b in range(BPC):
                    nc.gpsimd.dma_start(
                        yp_t[:], yp_ds[b][0:1, :].partition_broadcast(128).squeeze(1))
                    for g in range(NG):
                        xs = xp_t[:, (b * NBLK + g * G) * KC:(b * NBLK + (g + 1) * G) * KC]
                        xgk = xs.rearrange("p (g k) -> p g k", g=G)
                        x_apA = xgk[:, :, 0:2].unsqueeze(2).broadcast_to([128, G, N, 2])
                        x_apB = xgk[:, :, 2:3].unsqueeze(2).broadcast_to([128, G, N, 1])
                        # k slots 0,1 = (x0-y0), (x1-y1)
                        nc.vector.tensor_tensor(cgmk, x_apA, y_apA, op=A.subtract)
                        # k0 = d0^2 + d1^2
                        nc.vector._custom_dve(SQSQ, out=u_v, in0=u_v, in1=v_v)
                        # k1 = (x2-y2)
                        nc.vector.tensor_tensor(cB, x_apB, y_apB, op=A.subtract)
                        # k0 += k1^2  -> u
                        nc.vector._custom_dve(ADDSQ, out=u_v, in0=u_v, in1=v_v)
                        # row direction: min over m for each (p, g)
                        nc.vector.tensor_reduce(
                            rowacc[:, b * NBLK + g * G: b * NBLK + (g + 1) * G],
                            u_v, axis=X, op=A.min)
                        # col direction: min over the G blocks -> colstack slot g
                        nc.vector.tensor_reduce(
                            cs_t[:, g * N:(g + 1) * N],
                            c_t[:].rearrange("p (g q) -> p q g", g=G)[:, 0:2 * N, :]
                                 .rearrange("p (m k) g -> p k m g", k=2)[:, 0, :, :],
                            axis=X, op=A.min)
                    # min over the NG group slots -> per-batch column mins
                    nc.vector.tensor_reduce(
                        colout[:, b * N:(b + 1) * N],
                        cs_t[:].rearrange("p (g m) -> p m g", g=NG),
                        axis=X, op=A.min)

            nc.gpsimd.dma_start(col_d[:], colout[:])
            nc.gpsimd.dma_start(row_d[:], rowacc[:])
    nc.compile()
    return nc


def _build_operands(x, y):
    """x,y [B,N,3] f32 -> per-core input maps (f16 packed layouts)."""
    x = np.asarray(x, np.float32).astype(np.float16)
    y = np.asarray(y, np.float32).astype(np.float16)
    in_maps = []
    for core in range(NCORES):
        xp = np.empty((128, BPC * NBLK * KC), np.float16)
        maps = {}
        for j in range(BPC):
            bg = core * BPC + j
            # xp[p, (j*NBLK + r)*3 + k] = x[bg, r*128 + p, k]
            xb = x[bg].reshape(NBLK, 128, KC).transpose(1, 0, 2).reshape(128, NBLK * KC)
            xp[:, j * NBLK * KC:(j + 1) * NBLK * KC] = xb
            # yp[m*3+k] = y[bg, m, k]; broadcast to 128 partitions on-device
            maps[f"yp{j}"] = np.ascontiguousarray(y[bg].reshape(1, N * KC))
        maps["xp"] = xp
        in_maps.append(maps)
    return in_maps


_NC_CACHE = {}


def _get_nc(repeat: int = 1):
    if repeat not in _NC_CACHE:
        _NC_CACHE[repeat] = _build_nc(repeat)
    return _NC_CACHE[repeat]


def _finalize(results):
    total = 0.0
    for core in range(NCORES):
        row = np.asarray(results[core]["row"], np.float32)   # [128, BPC*NBLK]
        col = np.asarray(results[core]["col"], np.float32)   # [128, BPC*N]
        for j in range(BPC):
            rsum = row[:, j * NBLK:(j + 1) * NBLK].sum(dtype=np.float64)
            csum = col[:, j * N:(j + 1) * N].min(axis=0).sum(dtype=np.float64)
            total += (rsum + csum) / N
    return np.float32(total / B)


def kernel(x, y):
    x = np.asarray(x, dtype=np.float32)
    y = np.asarray(y, dtype=np.float32)
    assert x.shape == (B, N, KC) and y.shape == (B, N, KC)
    in_maps = _build_operands(x, y)
    nc = _get_nc(1)
    res = run_bass_kernel_spmd(nc, in_maps, core_ids=list(range(NCORES)))
    return _finalize(res.results)


# revision 22
# speedup vs baseline: 1.1359x; 1.1359x over previous
"""Chamfer distance kernel for Trainium2 (8 NeuronCores, data-parallel over batch).

Input : x, y float32 [16, 4096, 3]
Output: scalar float32 = mean_b [ mean_n min_m ||x_bn - y_bm||^2
                                + mean_m min_n ||x_bn - y_bm||^2 ]

This environment charges a large, roughly flat cost per *instruction*
(engines do not overlap), so the kernel minimizes instruction count by
computing squared distances directly on the Vector engine with giant
multi-dim access patterns (128 x 49152 elements per op, stride-0
broadcasts), instead of PE matmuls (capped at 512 columns per
instruction, which would need 512+ instructions per core).

Per core (2 batches, 32 x-blocks of 128 rows each, groups of G=4 blocks):
  c[p,k,g,m] = x_k[blk g, row p] - y_k[m]      1 TT sub (4D broadcast AP)
  u = c0^2 + c1^2 + c2^2                       2 custom DVE ops (SQSQ, ADDSQ)
  rowacc[p, blk] = min_m u                     1 segmented reduce
  colstack[g] = min over the G blocks          2 TT folds
After 8 groups: fold colstack 8->1 (3 TT), DMA out per-batch column mins
[128, 4096] f16 and row mins [128, 64] f32. The host does the final
128-partition min + mean (tiny numpy) because cross-partition DVE inputs
are illegal on this target and a DMA out is 1 instruction vs a 14-op tree.
y coordinates are loaded via a partition-broadcast DMA (24 KB upload per
batch instead of 3 MB).
"""
import sys

sys.path.insert(0, "/opt/trn_rl_repo")

import numpy as np

import concourse.bacc as bacc
import concourse.tile as tile
from concourse import mybir
from concourse.alu_op_type import AluOpType
from concourse.bass_utils import run_bass_kernel_spmd

# --- custom DVE ops (registered at import time) ---------------------------
import concourse.dve_ops as dve_ops
from concourse.dve_ops import DveOp
from concourse.dve_spec import Spec, Src0, Src1, sq, lower, _has_src1


def _register_dve_op(name, spec):
    if name in dve_ops._SUB_OPCODE_FOR_NAME:
        for o in dve_ops.OPS:
            if o.name == name:
                return o
    row = dve_ops._CUSTOM_DVE_ROW_BASE + len(dve_ops.OPS)
    assert row < 0x20
    dve_ops._SUB_OPCODE_FOR_NAME[name] = row
    from concourse.dve_uop import DveOpSpec

    shas = {}
    for ver in ("v3", "v4"):
        try:
            uops = lower(spec, ver=ver)
            s = DveOpSpec(name=name, opcode=row, uops=uops, rd1_en=_has_src1(spec))
            shas[ver] = s.sha(ver)
        except Exception:
            pass
    op = DveOp(name, spec, subdim=False, uops_sha=shas)
    dve_ops.OPS.append(op)
    dve_ops.CUSTOM_DVE_SPECS[name] = spec
    return op


SQSQ = _register_dve_op(
    "SQSQ_ANT",
    Spec(
        body=sq(Src0) + sq(Src1),
        reference=lambda in0, in1, s0, s1, imm2: (
            in0.astype(np.float32) ** 2 + in1.astype(np.float32) ** 2
        ),
    ),
)
ADDSQ = _register_dve_op(
    "ADDSQ_ANT",
    Spec(
        body=Src0 + sq(Src1),
        reference=lambda in0, in1, s0, s1, imm2: (
            in0.astype(np.float32) + in1.astype(np.float32) ** 2
        ),
    ),
)
# ---------------------------------------------------------------------------

F32 = mybir.dt.float32
F16 = mybir.dt.float16
X = mybir.AxisListType.X
A = AluOpType

B, N, KC = 16, 4096, 3
NCORES = 8
BPC = B // NCORES            # batches per core
NBLK = N // 128              # 32 x-blocks per batch
G = 8                        # blocks per group
NG = NBLK // G               # 4 groups per batch
GM = G * N                   # 32768 elements per k-plane
CW = 2 * GM                  # c holds two planes, interleaved at g granularity:
                             # c[p, g*2N + k*N + m], so every AP stride <= 8192
                             # (ISA step_elem and num_elem fields are 16-bit)


def _build_nc(repeat: int = 1):
    nc = bacc.Bacc("TRN2", target_bir_lowering=False, debug=False, num_devices=NCORES)
    xp_d = nc.dram_tensor("xp", [128, BPC * NBLK * KC], F16, kind="ExternalInput").ap()
    yp_ds = [
        nc.dram_tensor(f"yp{b}", [1, N * KC], F16, kind="ExternalInput").ap()
        for b in range(BPC)
    ]
    col_d = nc.dram_tensor("col", [128, BPC * N], F16, kind="ExternalOutput").ap()
    row_d = nc.dram_tensor("row", [128, BPC * NBLK], F32, kind="ExternalOutput").ap()

    with tile.TileContext(nc) as tc:
        import contextlib
        with contextlib.ExitStack() as ctx:
            const = ctx.enter_context(tc.tile_pool(name="const", bufs=1))

            xp_t = const.tile([128, BPC * NBLK * KC], F16, name="xp_t")
            nc.scalar.dma_start(xp_t[:], xp_d[:])
            yp_t = const.tile([128, N * KC], F16, name="yp_t")
            c_t = const.tile([128, CW], F16, name="c_t")
            cs_t = const.tile([128, NG * N], F16, name="cs_t")
            colout = const.tile([128, BPC * N], F16, name="colout")
            rowacc = const.tile([128, BPC * NBLK], F32, name="rowacc")

            # c layout [p, g, k, m]; sub iterates [p, k, g, m]
            c4 = c_t[:].rearrange("p (g k m) -> p k g m", g=G, k=2)
            u_v = c4[:, 0, :, :]                      # [128, G, N] k=0 subplanes
            v_v = c4[:, 1, :, :]                      # [128, G, N] k=1 subplanes
            cB = c4[:, 1:2, :, :]
            ykm = yp_t[:].rearrange("p (m k) -> p k m", k=KC)
            y_apA = ykm[:, 0:2, :].unsqueeze(2).broadcast_to([128, 2, G, N])
            y_apB = ykm[:, 2:3, :].unsqueeze(2).broadcast_to([128, 1, G, N])

            for _rep in range(repeat):
                for b in range(BPC):
                    nc.scalar.dma_start(
                        yp_t[:], yp_ds[b][0:1, :].partition_broadcast(128).squeeze(1))
                    for g in range(NG):
                        xs = xp_t[:, (b * NBLK + g * G) * KC:(b * NBLK + (g + 1) * G) * KC]
                        xkg = xs.rearrange("p (g k) -> p k g", g=G)
                        x_apA = xkg[:, 0:2, :].unsqueeze(3).broadcast_to([128, 2, G, N])
                        x_apB = xkg[:, 2:3, :].unsqueeze(3).broadcast_to([128, 1, G, N])
                        # subplanes 0,1 = (x0-y0), (x1-y1)
                        nc.vector.tensor_tensor(c4, x_apA, y_apA, op=A.subtract)
                        # subplane0 = d0^2 + d1^2
                        nc.vector._custom_dve(SQSQ, out=u_v, in0=u_v, in1=v_v)
                        # subplane1 = (x2-y2)
                        nc.vector.tensor_tensor(cB, x_apB, y_apB, op=A.subtract)
                        # subplane0 += subplane1^2  -> u
                        nc.vector._custom_dve(ADDSQ, out=u_v, in0=u_v, in1=v_v)
                        # row direction: min over m for each (p, g)
                        nc.vector.tensor_reduce(
                            rowacc[:, b * NBLK + g * G: b * NBLK + (g + 1) * G],
                            u_v, axis=X, op=A.min)
                        # col direction: min over the G blocks -> colstack slot g
                        nc.vector.tensor_reduce(
                            cs_t[:, g * N:(g + 1) * N],
                            c_t[:].rearrange("p (g k m) -> p k m g", g=G, k=2)[:, 0, :, :],
                            axis=X, op=A.min)
                    # min over the NG group slots -> per-batch column mins
                    nc.vector.tensor_reduce(
                        colout[:, b * N:(b + 1) * N],
                        cs_t[:].rearrange("p (g m) -> p m g", g=NG),
                        axis=X, op=A.min)

            nc.scalar.dma_start(col_d[:], colout[:])
            nc.scalar.dma_start(row_d[:], rowacc[:])
    nc.compile()
    return nc


def _build_operands(x, y):
    """x,y [B,N,3] f32 -> per-core input maps (f16 packed layouts)."""
    x = np.asarray(x, np.float32).astype(np.float16)
    y = np.asarray(y, np.float32).astype(np.float16)
    in_maps = []
    for core in range(NCORES):
        xp = np.empty((128, BPC * NBLK * KC), np.float16)
        maps = {}
        for j in range(BPC):
            bg = core * BPC + j
            # xp[p, (j*NBLK + r)*3 + k] = x[bg, r*128 + p, k]
            xb = x[bg].reshape(NBLK, 128, KC).transpose(1, 0, 2).reshape(128, NBLK * KC)
            xp[:, j * NBLK * KC:(j + 1) * NBLK * KC] = xb
            # yp[m*3+k] = y[bg, m, k]; broadcast to 128 partitions on-device
            maps[f"yp{j}"] = np.ascontiguousarray(y[bg].reshape(1, N * KC))
        maps["xp"] = xp
        in_maps.append(maps)
    return in_maps


_NC_CACHE = {}


def _get_nc(repeat: int = 1):
    if repeat not in _NC_CACHE:
        _NC_CACHE[repeat] = _build_nc(repeat)
    return _NC_CACHE[repeat]


def _finalize(results):
    total = 0.0
    for core in range(NCORES):
        row = np.asarray(results[core]["row"], np.float32)   # [128, BPC*NBLK]
        col = np.asarray(results[core]["col"], np.float32)   # [128, BPC*N]
        for j in range(BPC):
            rsum = row[:, j * NBLK:(j + 1) * NBLK].sum(dtype=np.float64)
            csum = col[:, j * N:(j + 1) * N].min(axis=0).sum(dtype=np.float64)
            total += (rsum + csum) / N
    return np.float32(total / B)


def kernel(x, y):
    x = np.asarray(x, dtype=np.float32)
    y = np.asarray(y, dtype=np.float32)
    assert x.shape == (B, N, KC) and y.shape == (B, N, KC)
    in_maps = _build_operands(x, y)
    nc = _get_nc(1)
    res = run_bass_kernel_spmd(nc, in_maps, core_ids=list(range(NCORES)))
    return _finalize(res.results)
